# revision 15
# baseline (speedup 1.0000x reference)
"""3-layer GCN + mean-pool + classifier for Trainium2, SPMD on 8 NeuronCores.

Self-contained: kernel(**inputs) takes the full-size numpy inputs, does the
host-side graph partitioning, builds/compiles a Bass/Tile kernel, runs it on
cores 0-7 via run_bass_kernel_spmd, and returns the [128, 3] log-softmax
output.

Distribution: nodes are dst-sharded across the 8 cores. Per GCN layer each
core computes t' = dinv * (h @ W) for its shard (TensorE), stores it as
256B bf16 rows (64 feats + 64 zero pad), AllGathers the shards into a full
node-major table in DRAM, and dma_gathers its in-edges' source rows. The
scatter-add of the previous design is replaced by TensorE accumulation:
edges are sorted by (dst-slab, src-bucket, dst), segments are padded to the
cross-core max so the call/window/run structure is SPMD-static, and each
128-slot window is reduced into per-chunk PSUM accumulators with one-hot
lhsT matrices built on DVE (iota==runvec compare). Self-loops never touch
the edge path: the t' staging tile is added to the PSUM result directly.
The GCN normalization deg^-1/2 (A+I) deg^-1/2 factorizes into a pre-scale
of t' and a post-scale of the aggregate. Mean-pooling runs as a one-hot
matmul on TensorE with an AllReduce of per-core partials; the classifier +
log_softmax run replicated on every core.

HW limits (empirical): gather idx values must be < 8192 (16 source
buckets), gather calls <= 1024 indices. Pad gather slots use index 0 (mid-
call -1 is unsafe); their one-hot rows are 255 so they contribute zero.
"""
import sys

sys.path.insert(0, "/opt/trn_rl_repo")

import numpy as np
import ml_dtypes
import concourse.bacc as bacc
import concourse.mybir as mybir
import concourse.tile as tile
from concourse.masks import make_identity
import concourse.tile as _tile
import concourse.mybir as _mybir
from concourse.vector_clock import ScopedClock as _ScopedClock

# ---------------------------------------------------------------------------
# Workarounds: this walrus build rejects >1 sync-wait per instruction.


def _split_waits_tail(nc, inst):
    si = inst.ins.sync_info
    if si is None or not si.on_wait or len(si.on_wait) <= 1:
        return
    waits = list(si.on_wait)
    inst.ins.sync_info = _mybir.SyncInfo(on_wait=[], on_update=list(si.on_update or []))
    for w in waits:
        nop = nc.sync.nop()
        nop.ins.sync_info = _mybir.SyncInfo(on_wait=[w], on_update=[])


def _drain_and_barrier(self, tick_clock, wait_clock):
    nc = self.nc
    probe = nc.sync.nop()
    wait_clock.add_sem_waits(probe.ins, _ScopedClock({None: tick_clock.global_clock}))
    _split_waits_tail(nc, probe)
    nc.sync.drain()
    nc.all_engine_barrier()
    assert self.sems is not None
    popped = nc._tile_sem_poison_stack.pop()
    assert popped is self._sem_poison
    nc.clear_and_free_semaphores(list(self.sems.allocated().values()))
    nc.all_engine_barrier()


_tile.TileContext._drain_and_barrier = _drain_and_barrier


def fix_multiwait(nc):
    """Rewrite every >1-wait instruction into wait-nops + 1-wait instruction."""
    for f in nc.m.functions:
        for blk in f.blocks:
            insts = blk.instructions            # live list (rust-backed)
            i = 0
            while i < len(insts):
                inst = insts[i]
                si = inst.sync_info
                if si is not None and si.on_wait and len(si.on_wait) > 1:
                    waits = list(si.on_wait)
                    eng = inst.engine
                    inst.sync_info = _mybir.SyncInfo(
                        on_wait=[waits[-1]], on_update=list(si.on_update or [])
                    )
                    for j, w in enumerate(waits[:-1]):
                        nop = nc.engines[eng].nop(hint="mwfix")
                        popped = False
                        for f2 in nc.m.functions:
                            for b2 in f2.blocks:
                                l2 = b2.instructions
                                if l2 and l2[-1].name == nop.ins.name:
                                    l2.pop()
                                    popped = True
                                    break
                            if popped:
                                break
                        assert popped, "could not relocate mwfix nop"
                        nop.ins.sync_info = _mybir.SyncInfo(on_wait=[w], on_update=[])
                        insts.insert(i + j, nop.ins)
                    i += len(waits) - 1
                i += 1


# ---------------------------------------------------------------------------

F32 = mybir.dt.float32
BF16 = mybir.dt.bfloat16
I16 = mybir.dt.int16
AF = mybir.ActivationFunctionType
ALU = mybir.AluOpType
BF16NP = ml_dtypes.bfloat16


def cdiv(a, b):
    return (a + b - 1) // b


def rup(a, b):
    return cdiv(a, b) * b


class Cfg:
    def __init__(self, N, E, IN, HID, G, OUT):
        self.C = 8
        self.N, self.E, self.IN, self.HID, self.G, self.OUT = N, E, IN, HID, G, OUT
        assert N % self.C == 0
        self.NSH = N // self.C            # 12500
        self.TROW = rup(self.NSH, 128)    # 12544
        self.NCHK = self.TROW // 128      # 98
        self.NBUK = 16
        assert (self.C * self.TROW) % self.NBUK == 0
        self.SRCW = self.C * self.TROW // self.NBUK   # 6272
        assert self.SRCW <= 8191
        self.S = 7                        # chunks per slab
        assert self.NCHK % self.S == 0
        self.NSLAB = self.NCHK // self.S  # 14
        self.EL = 128                     # bf16 elems per table row (256B)
        self.MAXG = 1024
        assert G <= 128


def prep(inputs, cfg):
    c = cfg
    x = np.asarray(inputs["x"], np.float32)
    ei = np.asarray(inputs["edge_index"], np.int64)
    batch = np.asarray(inputs["batch"], np.int64)
    W1 = np.asarray(inputs["W1"], np.float32); b1 = np.asarray(inputs["b1"], np.float32)
    W2 = np.asarray(inputs["W2"], np.float32); b2 = np.asarray(inputs["b2"], np.float32)
    W3 = np.asarray(inputs["W3"], np.float32); b3 = np.asarray(inputs["b3"], np.float32)
    Wc = np.asarray(inputs["Wc"], np.float32); bc = np.asarray(inputs["bc"], np.float32)

    src = ei[0].astype(np.int64)
    dst = ei[1].astype(np.int64)
    deg = np.bincount(dst, minlength=c.N).astype(np.float32) + 1.0
    dinv = 1.0 / np.sqrt(deg)

    HID = c.HID
    W3p = np.zeros((HID, HID), np.float32); W3p[:, : W3.shape[1]] = W3
    b3p = np.zeros((HID,), np.float32); b3p[: b3.shape[0]] = b3
    Wcp = np.zeros((HID, c.OUT), np.float32); Wcp[: Wc.shape[0]] = Wc

    core_of = src // c.NSH
    dcore = dst // c.NSH
    src_local = src - core_of * c.NSH

    # Streams: per (slab, bkey) where bkey 0..NBUK-1 = remote bucket (agout
    # rows [b*SRCW,(b+1)*SRCW)), bkey NBUK+lb (lb in 0,1) = local half (agin
    # rows [lb*SRCW,(lb+1)*SRCW)). Edges with src_core == dst_core are local.
    # Within a stream edges are sorted by dst; per-core counts differ, the
    # program uses the cross-core max, shorter cores pad with idx -1 (trailing
    # pads are trimmed by the gather ucode before descriptor generation).
    NLB = c.NBUK + 2
    per_core = []   # [ci][(s, bkey)] = (gidx, dloc)
    cnt = np.zeros((c.C, c.NSLAB, NLB), np.int64)
    for ci in range(c.C):
        m = dcore == ci
        dl = dst[m] - ci * c.NSH
        sl = dl // (128 * c.S)
        is_loc = core_of[m] == ci
        s_loc = src_local[m]
        trow = core_of[m] * c.TROW + s_loc
        bkey = np.where(is_loc, c.NBUK + s_loc // c.SRCW, trow // c.SRCW)
        gg = np.where(is_loc, s_loc % c.SRCW, trow % c.SRCW)
        order = np.lexsort((dl, bkey, sl))
        dl, gg, bkey, sl = dl[order], gg[order], bkey[order], sl[order]
        key = sl * NLB + bkey
        d = {}
        bounds = np.r_[0, np.flatnonzero(np.diff(key)) + 1, len(key)]
        for i in range(len(bounds) - 1):
            a, b = bounds[i], bounds[i + 1]
            d[(int(sl[a]), int(bkey[a]))] = (gg[a:b], dl[a:b])
            cnt[ci, sl[a], bkey[a]] = b - a
        per_core.append(d)
    SBMAX = cnt.max(axis=0)  # [NSLAB, NLB]

    # Static program layout. Local streams (bkey >= NBUK) come first per slab.
    calls = []
    nruns = 0
    col = 0
    for s in range(c.NSLAB):
        for bkey in list(range(c.NBUK, NLB)) + list(range(c.NBUK)):
            n_sb = int(SBMAX[s, bkey])
            if n_sb == 0:
                continue
            off0 = 0
            while off0 < n_sb:
                n = min(c.MAXG, rup(n_sb - off0, 128))
                # union of chunks any core has in each window
                runs = []
                for w in range(n // 128):
                    lo, hi = off0 + w * 128, off0 + w * 128 + 128
                    cmin, cmax = None, None
                    for ci in range(c.C):
                        cc = int(cnt[ci, s, bkey])
                        if cc <= lo:
                            continue
                        dlc = per_core[ci].get((s, bkey))
                        if dlc is None:
                            continue
                        dl = dlc[1]
                        a = dl[lo] // 128
                        bql = dl[min(hi, cc) - 1] // 128
                        cmin = a if cmin is None else min(cmin, a)
                        cmax = bql if cmax is None else max(cmax, bql)
                    if cmin is None:
                        continue
                    for ch in range(int(cmin), int(cmax) + 1):
                        runs.append([w, int(ch), 0, 0, nruns, False, False])
                        nruns += 1
                calls.append(dict(slab=s, bkey=bkey, n=n, col=col, off0=off0,
                                  runs=runs))
                col += n
                off0 += n
    TOTSLOT = col
    first_seen, last_seen = {}, {}
    for call in calls:
        for r in call["runs"]:
            ch = r[1]
            if ch not in first_seen:
                first_seen[ch] = r
            last_seen[ch] = r
    for r in first_seen.values():
        r[5] = True
    for r in last_seen.values():
        r[6] = True
    assert len(first_seen) == c.NCHK, \
        f"chunks with no runs: {set(range(c.NCHK)) - set(first_seen)}"

    cntg = np.bincount(batch, minlength=c.G).astype(np.float32)
    cntinv = (1.0 / np.maximum(cntg, 1.0)).astype(np.float32)

    in_maps = []
    for ci in range(c.C):
        lo, hi = ci * c.NSH, (ci + 1) * c.NSH
        xT = np.zeros((c.IN, c.TROW), np.float32)
        xT[:, : c.NSH] = x[lo:hi].T
        dv = np.zeros((c.TROW,), np.float32)
        dv[: c.NSH] = dinv[lo:hi]

        g_slots = np.zeros(TOTSLOT, np.int64)   # PADMODE0: all pads real idx 0
        runvecs = np.full((128, max(nruns, 1)), 255.0, np.float32)
        segs = per_core[ci]
        for call in calls:
            s, bkey, n, col0, off0 = (call["slab"], call["bkey"], call["n"],
                                      call["col"], call["off0"])
            cc = int(cnt[ci, s, bkey])
            ge, de = segs.get((s, bkey), (None, None))
            nreal = min(max(cc - off0, 0), n)
            if nreal > 0:
                g_slots[col0:col0 + nreal] = ge[off0:off0 + nreal]
            # >=16 real descs per call so all 16 SDMA sem increments fire
            nmin = min(16, n)
            if nreal < nmin:
                g_slots[col0 + nreal:col0 + nmin] = 0
            for w, ch, _, _, rid, _, _ in call["runs"]:
                lo_w = off0 + w * 128
                hi_w = min(off0 + w * 128 + 128, cc)
                if hi_w <= lo_w:
                    continue
                dl = de[lo_w:hi_w]
                sel = (dl // 128) == ch
                pvec = np.full(128, 255.0, np.float32)
                pvec[: hi_w - lo_w][sel] = (dl[sel] % 128).astype(np.float32)
                runvecs[:, rid] = pvec
        gidx_w = np.tile(
            g_slots.astype(np.int16).reshape(-1, 16).T, (8, 1)).astype(np.int16)
        oh_all = (runvecs.T[:, :, None] ==
                  np.arange(128, dtype=np.float32)[None, None, :])
        oh_all = oh_all.reshape(nruns * 128, 128).astype(BF16NP)
        dinvrep = np.repeat(dv.reshape(c.NCHK, 128).T[:, :, None], 64, axis=2
                            ).reshape(128, c.NCHK * 64)
        b3rep_w = np.tile(b3p[None, None, :], (128, c.NCHK, 1)).reshape(
            128, c.NCHK * 64)

        oneh = np.zeros((c.TROW, 128), np.float32)
        oneh[np.arange(c.NSH), batch[lo:hi].astype(np.int64)] = 1.0

        bcols = np.stack([b1, b2, b3p], axis=1)
        bcrep = np.tile(bc[None, :], (128, 1))
        cinv = np.zeros((128, 1), np.float32)
        cinv[: c.G, 0] = cntinv

        in_maps.append(dict(
            xT=xT, gidx=gidx_w, ohruns=oh_all,
            dinvrep=dinvrep.astype(BF16NP),
            b3repw=b3rep_w.astype(BF16NP),
            oneh=oneh.astype(BF16NP),
            W1d=W1, W2d=W2.astype(BF16NP), W3d=W3p.astype(BF16NP),
            bcols=bcols, Wcp=Wcp, bcrep=bcrep, cinv=cinv,
        ))

    maxruns = max((len(cl["runs"]) for cl in calls), default=1)
    meta = dict(calls=calls, nruns=nruns, TOTSLOT=TOTSLOT, MAXRUNS=maxruns)
    return in_maps, meta


def build(cfg, meta):
    c = cfg
    HID, G, OUT, EL = c.HID, c.G, c.OUT, c.EL
    calls, NRUNS, TOTSLOT = meta["calls"], meta["nruns"], meta["TOTSLOT"]
    MAXRUNS = meta["MAXRUNS"]

    nc = bacc.Bacc("TRN2", num_devices=c.C, dynamic_dma_scratch_size=16384)

    def ein(name, shape, dt=F32):
        return nc.dram_tensor(name, shape, dt, kind="ExternalInput")

    xT_d = ein("xT", [c.IN, c.TROW])
    gidx_d = ein("gidx", [128, TOTSLOT // 16], I16)
    ohruns_d = ein("ohruns", [NRUNS * 128, 128], BF16)
    dinvrep_d = ein("dinvrep", [128, c.NCHK * HID], BF16)
    b3repw_d = ein("b3repw", [128, c.NCHK * HID], BF16)
    oneh_d = ein("oneh", [c.TROW, 128], BF16)
    W1_d = ein("W1d", [c.IN, HID])
    W2_d = ein("W2d", [HID, HID], BF16)
    W3_d = ein("W3d", [HID, HID], BF16)
    bcols_d = ein("bcols", [HID, 3])
    Wc_d = ein("Wcp", [HID, OUT])
    bcrep_d = ein("bcrep", [128, OUT])
    cinv_d = ein("cinv", [128, 1])

    agin_d = nc.dram_tensor("agin", [c.TROW, EL], BF16, kind="Internal")
    agout_d = nc.dram_tensor(
        "agout", [c.C * c.TROW, EL], BF16, kind="Internal", addr_space="Shared")
    plin_d = nc.dram_tensor("plin", [128, HID], F32, kind="Internal")
    plout_d = nc.dram_tensor(
        "plout", [128, HID], F32, kind="Internal", addr_space="Shared")
    y_d = nc.dram_tensor("y", [G, OUT], F32, kind="ExternalOutput")

    rg = [list(range(c.C))]

    with tile.TileContext(nc) as tc:
        with (
            tc.tile_pool(name="res", bufs=1) as res,
            tc.tile_pool(name="stage", bufs=1) as stpool,
            tc.tile_pool(name="work", bufs=8) as work,
            tc.tile_pool(name="ohp", bufs=8) as ohp,
            tc.tile_pool(name="msgs", bufs=6) as msgs,
            tc.tile_pool(name="psA", bufs=2, space="PSUM") as psA,
            tc.tile_pool(name="psB", bufs=1, space="PSUM") as psB,
            tc.tile_pool(name="psC", bufs=2, space="PSUM") as psC,
        ):
            ident = res.tile([128, 128], F32)
            make_identity(nc, ident[:])
            dinvrep_sb = res.tile([128, c.NCHK, HID], BF16)
            nc.sync.dma_start(
                dinvrep_sb[:].rearrange("p a b -> p (a b)"), dinvrep_d[:])
            b3repw_sb = res.tile([128, c.NCHK, HID], BF16)
            nc.sync.dma_start(
                b3repw_sb[:].rearrange("p a b -> p (a b)"), b3repw_d[:])
            W1_sb = res.tile([c.IN, HID], F32, name="w1")
            nc.sync.dma_start(W1_sb[:], W1_d[:])
            W2_sb = res.tile([HID, HID], BF16, name="w2")
            nc.sync.dma_start(W2_sb[:], W2_d[:])
            W3_sb = res.tile([HID, HID], BF16, name="w3")
            nc.sync.dma_start(W3_sb[:], W3_d[:])
            bcols_sb = res.tile([HID, 3], F32)
            nc.sync.dma_start(bcols_sb[:], bcols_d[:])
            Wc_sb = res.tile([HID, OUT], F32)
            nc.sync.dma_start(Wc_sb[:], Wc_d[:])
            bcrep_sb = res.tile([128, OUT], F32)
            nc.sync.dma_start(bcrep_sb[:], bcrep_d[:])
            cinv_sb = res.tile([128, 1], F32)
            nc.sync.dma_start(cinv_sb[:], cinv_d[:])

            hT_sb = stpool.tile([HID, c.TROW], BF16)
            stage2_sb = stpool.tile([128, c.NCHK, HID], F32)
            stagebf_sb = stpool.tile([128, c.NCHK, HID], BF16)
            h3_sb = stpool.tile([128, c.NCHK, HID], BF16)

            agin_r = agin_d[:].rearrange("(k p) f -> p k f", p=128)
            # zero agin's upper 64-col half once (table rows are 256B)
            nc.vector.memset(stagebf_sb[:], 0.0)
            nc.sync.dma_start(agin_r[:, :, HID:], stagebf_sb[:])

            pp = psA.tile([128, HID], F32, space="PSUM", tag="pool", bufs=1)

            nreg = nc.gpsimd.alloc_register("nidx")
            _regval = [None]

            def set_nreg(v):
                if _regval[0] != v:
                    nc.gpsimd.reg_mov(nreg, v)
                    _regval[0] = v

            OCT = 7
            for l in range(3):
                K = c.IN if l == 0 else HID
                W_sb = [W1_sb, W2_sb, W3_sb][l]
                # ---- phase 1: t' = dinv * (h @ W), oct-batched ----
                for ko in range(0, c.NCHK, OCT):
                    ke = min(c.NCHK, ko + OCT)
                    ps = psA.tile([128, OCT, HID], F32, space="PSUM")
                    for k in range(ko, ke):
                        if l == 0:
                            xt = work.tile([c.IN, 128], F32, tag="xt")
                            nc.sync.dma_start(
                                xt[:], xT_d[:, k * 128:(k + 1) * 128])
                            lhsT = xt[:, :]
                        else:
                            lhsT = hT_sb[:K, k * 128:(k + 1) * 128]
                        nc.tensor.matmul(ps[:, k - ko, :], lhsT, W_sb[:K, :],
                                         start=True, stop=True)
                    nb = ke - ko
                    nc.vector.tensor_tensor(
                        stagebf_sb[:, ko:ke, :], ps[:, :nb, :],
                        dinvrep_sb[:, ko:ke, :], ALU.mult)
                    nc.vector.tensor_tensor(
                        stage2_sb[:, ko:ke, :], ps[:, :nb, :],
                        dinvrep_sb[:, ko:ke, :], ALU.mult)
                    nc.vector.tensor_tensor(
                        stage2_sb[:, ko:ke, :], stage2_sb[:, ko:ke, :],
                        dinvrep_sb[:, ko:ke, :], ALU.mult)
                    if l == 2:
                        nc.vector.tensor_tensor(
                            stage2_sb[:, ko:ke, :], stage2_sb[:, ko:ke, :],
                            b3repw_sb[:, ko:ke, :], ALU.add)
                    nc.sync.dma_start(
                        agin_r[:, ko:ke, :HID], stagebf_sb[:, ko:ke, :])
                nc.gpsimd.collective_compute(
                    "AllGather", ALU.bypass,
                    replica_groups=rg, ins=[agin_d[:]], outs=[agout_d[:]])

                # ---- phase 2: edge pass, slab by slab ----
                pchunk = {}
                slab_ps = [None]
                cur_slab = -1

                def flush_slab(slab):
                    if slab_ps[0] is None:
                        pchunk.clear()
                        return
                    s0 = slab * c.S
                    v2 = work.tile([128, c.S, HID], F32, tag="v2")
                    nc.vector.tensor_tensor(
                        v2[:], slab_ps[0][:], dinvrep_sb[:, s0:s0 + c.S, :],
                        ALU.mult)
                    nc.vector.tensor_tensor(
                        v2[:], v2[:], stage2_sb[:, s0:s0 + c.S, :], ALU.add)
                    if l < 2:
                        for cc in range(c.S):
                            ch = s0 + cc
                            psT = psB.tile([HID, 128], F32, space="PSUM")
                            nc.tensor.transpose(psT[:], v2[:, cc, :], ident[:])
                            nc.scalar.activation(
                                hT_sb[:, ch * 128:(ch + 1) * 128], psT[:],
                                AF.Relu, bias=bcols_sb[:, l:l + 1])
                    else:
                        nc.scalar.activation(
                            h3_sb[:, s0:s0 + c.S, :], v2[:], AF.Relu)
                        for cc in range(c.S):
                            ch = s0 + cc
                            oh2 = work.tile([128, 128], BF16, tag="oh2")
                            nc.sync.dma_start(
                                oh2[:], oneh_d[ch * 128:(ch + 1) * 128, :])
                            nc.tensor.matmul(
                                pp[:], oh2[:], h3_sb[:, ch, :],
                                start=(ch == 0), stop=(ch == c.NCHK - 1))
                    pchunk.clear()

                for call in calls:
                    s, bk, n, col0 = (call["slab"], call["bkey"],
                                      call["n"], call["col"])
                    if s != cur_slab:
                        flush_slab(cur_slab)
                        slab_ps[0] = None
                        cur_slab = s
                    gi = work.tile([128, c.MAXG // 16], I16, tag="gi",
                                   name=f"gi_{l}_{col0}")
                    nc.sync.dma_start(
                        gi[:, : n // 16],
                        gidx_d[:, col0 // 16:(col0 + n) // 16])
                    msg = msgs.tile([128, c.MAXG // 128, EL], BF16, tag="msg",
                                    name=f"msg_{l}_{col0}")
                    set_nreg(n)
                    if bk >= c.NBUK:
                        lb = bk - c.NBUK
                        src_ap = agin_d[lb * c.SRCW:(lb + 1) * c.SRCW, :]
                    else:
                        src_ap = agout_d[bk * c.SRCW:(bk + 1) * c.SRCW, :]
                    nc.gpsimd.dma_gather(
                        msg[:, : n // 128, :], src_ap,
                        gi[:, : n // 16], n, nreg, EL)
                    nr = len(call["runs"])
                    if nr:
                        rid0 = call["runs"][0][4]
                        oht = ohp.tile([128, MAXRUNS, 128], BF16, tag="oht",
                                       name=f"oht_{l}_{col0}")
                        nc.sync.dma_start(
                            oht[:, :nr, :],
                            ohruns_d[rid0 * 128:(rid0 + nr) * 128, :]
                            .rearrange("(r p) f -> p r f", p=128))
                    for w, ch, a, e, rid, first, lastf in call["runs"]:
                        if first:
                            assert ch not in pchunk
                            if slab_ps[0] is None:
                                slab_ps[0] = psC.tile(
                                    [128, c.S, HID], F32, space="PSUM",
                                    tag="pc", name=f"pc_{l}_{s}")
                            cc = ch - s * c.S
                            pchunk[ch] = slab_ps[0][:, cc, :]
                        nc.tensor.matmul(
                            pchunk[ch], oht[:, rid - rid0, :],
                            msg[:, w, :HID],
                            start=first, stop=lastf)
                flush_slab(cur_slab)

            # ---- mean pool tail ----
            pl = res.tile([128, HID], F32)
            nc.vector.tensor_copy(pl[:], pp[:])
            nc.sync.dma_start(plin_d[:], pl[:])
            nc.gpsimd.collective_compute(
                "AllReduce", ALU.add,
                replica_groups=rg, ins=[plin_d[:]], outs=[plout_d[:]])
            plr = res.tile([128, HID], F32)
            nc.sync.dma_start(plr[:], plout_d[:])
            plm = res.tile([128, HID], F32)
            nc.vector.tensor_scalar_mul(plm[:], plr[:], cinv_sb[:])
            psT = psB.tile([HID, 128], F32, space="PSUM", tag="pT", bufs=1)
            nc.tensor.transpose(psT[:], plm[:], ident[:])
            plT = res.tile([HID, 128], F32)
            nc.vector.tensor_copy(plT[:], psT[:])
            psD = psB.tile([G, OUT], F32, space="PSUM", tag="pC", bufs=1)
            nc.tensor.matmul(psD[:], plT[:, :G], Wc_sb[:], start=True, stop=True)
            lg = res.tile([G, OUT], F32)
            nc.vector.tensor_add(lg[:], psD[:, :], bcrep_sb[:G, :])
            mx = res.tile([G, 1], F32)
            nc.vector.tensor_reduce(mx[:], lg[:], mybir.AxisListType.X, ALU.max)
            lgs = res.tile([G, OUT], F32)
            nc.vector.tensor_scalar_sub(lgs[:], lg[:], mx[:])
            ex = res.tile([G, OUT], F32)
            nc.scalar.activation(ex[:], lgs[:], AF.Exp)
            sm = res.tile([G, 1], F32)
            nc.vector.tensor_reduce(sm[:], ex[:], mybir.AxisListType.X, ALU.add)
            ls = res.tile([G, 1], F32)
            nc.scalar.activation(ls[:], sm[:], AF.Ln)
            yt = res.tile([G, OUT], F32)
            nc.vector.tensor_scalar_sub(yt[:], lgs[:], ls[:])
            nc.sync.dma_start(y_d[:], yt[:])

    return nc


def _finalize(nc):
    nc.compile()
    fix_multiwait(nc)


def run(inputs, cfg, profile_dir=None):
    from concourse.bass_utils import run_bass_kernel_spmd

    in_maps, meta = prep(inputs, cfg)
    nc = build(cfg, meta)
    _finalize(nc)
    if profile_dir is not None:
        from trn_agent_boot.trn_boot import _ntff_profile_via_ctypes
        hook = _ntff_profile_via_ctypes("/opt/axon/libaxon_pjrt.so")
        with hook(profile_dir, [0]):
            res = run_bass_kernel_spmd(nc, in_maps, core_ids=list(range(cfg.C)))
    else:
        res = run_bass_kernel_spmd(nc, in_maps, core_ids=list(range(cfg.C)))
    return res.results[0]["y"]


# ---------------------------------------------------------------------------
N_NODES, N_EDGES, IN_DIM, HID_DIM, N_GRAPHS, OUT_DIM = 100_000, 1_600_000, 128, 64, 128, 3


def kernel(**inputs):
    import os
    cfg = Cfg(N_NODES, N_EDGES, IN_DIM, HID_DIM, N_GRAPHS, OUT_DIM)
    out = run(inputs, cfg, profile_dir=os.environ.get("GNN_PROFILE_DIR"))
    return np.asarray(out, np.float32)
